# revision 32
# baseline (speedup 1.0000x reference)
"""Trainium2 Bass kernel for nn_CovarianceLayer: local 5x5 covariance of two images.

reference:
    xc = x[:, :, 2:-2, 2:-2]; yc likewise
    x_dev = xc - box5x5(x)/25 ; y_dev = yc - box5x5(y)/25
    out   = box5x5(x_dev * y_dev)/25            # [B,1,1016,1016]

Strategy (pure data parallel over batch, 2 images per NeuronCore, 8 cores):
  Per 128-row block (stride 120, 8-row vertical halo):
    - horizontal 5-tap box sums of x,y on DVE via a custom prefix-scan op
    - vertical 5-tap conv + center-crop subtraction fused into PE matmuls
      (f32r, full-rate): PSUM = Wid^T @ x_shift  -  (band/32)^T @ hx
    - ScalarE drains dev PSUM to bf16 SBUF (one [*,1020] drain per pane)
    - p = x_dev * y_dev and q[c] = p[c] + p[c+2] on DVE (bf16, 2x mode)
    - final 5x5 box: horizontal 5-sum decomposed as q@0 + q@1 + p@4, each
      vertically convolved by a band matmul accumulating into one PSUM
    - PSUM -> SBUF copies on ScalarE; loads on SP HWDGE ring, stores on the
      ACT ring (so stores never queue ahead of prefetch loads)
"""

import numpy as np

import concourse.bacc as bacc
import concourse.mybir as mybir
import concourse.tile as tile
import concourse.dve_ops as dve_ops
from concourse.dve_spec import Spec, Src0, Src1, C0, scan, AluOp, lower
from concourse.dve_uop import DveOpSpec
from concourse.dve_ops import DveOp
from concourse import bass_utils

dt = mybir.dt

H = W = 1024
HO = WO = 1016   # output spatial dims
HD = WD = 1020   # x_dev dims
B_PER_CORE = 2
N_CORES = 8
BLK = 120        # output rows per block


def _register_box5():
    """out[p,k] = sum_{d=0..4} v[p,k+d]; in0=v[:,4:4+N], in1=v[:,0:N], s0=sum(v[:,0:4])."""
    name = "BOX5_ANT"
    for op in dve_ops.OPS:
        if op.name == name:
            return op
    body = scan(AluOp.ADD, Src0 - Src1, init=C0) + Src1

    def ref(in0, in1, c0, c1, c2):
        return np.cumsum(in0 - in1, axis=-1, dtype=np.float32) + in1 + c0

    spec = Spec(body=body, reference=ref)
    row = dve_ops._CUSTOM_DVE_ROW_BASE + len(dve_ops.OPS)
    shas = {}
    for ver in ("v3", "v4"):
        uops = lower(spec, ver=ver)
        shas[ver] = DveOpSpec(name=name, opcode=row, uops=uops, rd1_en=True).sha(ver)
    op = DveOp(name, spec, subdim=False, uops_sha=shas)
    dve_ops.OPS.append(op)
    dve_ops.CUSTOM_DVE_SPECS[name] = spec
    dve_ops._SUB_OPCODE_FOR_NAME[name] = row
    return op


# psum_dev = (25/32)*xc - box2D(x)/32 = (25/32) * x_dev  (exact weights)
# p = (25/32)^2 * x_dev*y_dev (bf16); out' = box2D(p)/16; out = out' * OUT_SCALE
WID_V = 25.0 / 32.0
OUT_SCALE = 16.0 / (25.0 * WID_V * WID_V)  # = 1.048576


def _make_weights():
    import ml_dtypes
    # Wid[k, m] = 25/32 iff k == m+2   (center-crop tap, same PSUM accum group)
    # Wnb[k, m] = -1/32 iff m <= k <= m+4  (negated vertical band, bf16)
    # Wpb[k, m] = +1/16 iff m <= k <= m+4  (final vertical band, bf16)
    wid = np.zeros((128, 128), dtype=np.float32)
    wnb = np.zeros((128, 128), dtype=np.float32)
    for m in range(124):
        wid[m + 2, m] = WID_V
        wnb[m:m + 5, m] = -1.0 / 32.0
    wpb = np.zeros((124, 128), dtype=np.float32)
    for m in range(120):
        wpb[m:m + 5, m] = 1.0 / 16.0
    return wid, wnb.astype(ml_dtypes.bfloat16), wpb.astype(ml_dtypes.bfloat16)


def _hsum_into(nc, box5, sb, out_tile, src_tile, rows, n_in, tag, o0=0, s0c=0):
    """out_tile[0:rows, o0:o0+n_in-4] = horizontal 5-tap box sums of
    src_tile[0:rows, s0c:s0c+n_in]."""
    s3 = sb.tile([128, 1], dt.float32, tag=f"s3_{tag}")
    nc.vector.tensor_reduce(s3[0:rows, :], src_tile[0:rows, s0c:s0c + 4],
                            op=mybir.AluOpType.add, axis=mybir.AxisListType.X)
    n_out = n_in - 4
    nc.vector._custom_dve(box5, out=out_tile[0:rows, o0:o0 + n_out],
                          in0=src_tile[0:rows, s0c + 4:s0c + n_in],
                          in1=src_tile[0:rows, s0c:s0c + n_out],
                          s0=s3[0:rows, :])


def build_bass():
    box5 = _register_box5()
    nc = bacc.Bacc("TRN2", target_bir_lowering=False)

    x_d = nc.dram_tensor("x", [B_PER_CORE, H, W], dt.float32r, kind="ExternalInput")
    y_d = nc.dram_tensor("y", [B_PER_CORE, H, W], dt.float32r, kind="ExternalInput")
    wid_d = nc.dram_tensor("wid", [128, 128], dt.float32r, kind="ExternalInput")
    wnb_d = nc.dram_tensor("wnb", [128, 128], dt.bfloat16, kind="ExternalInput")
    wpb_d = nc.dram_tensor("wpb", [124, 128], dt.bfloat16, kind="ExternalInput")
    o_d = nc.dram_tensor("o", [B_PER_CORE, HO, WO], dt.float32, kind="ExternalOutput")

    n_blocks = (HO + BLK - 1) // BLK  # 9
    n_total = n_blocks

    def block_geom(i):
        r0 = BLK * i
        return (r0, min(128, H - r0), min(124, HD - r0), min(BLK, HO - r0))

    with tile.TileContext(nc) as tc:
        with tc.tile_pool(name="wts", bufs=1) as wts, \
             tc.tile_pool(name="sb4", bufs=4) as sb4, \
             tc.tile_pool(name="sb3", bufs=3) as sb3, \
             tc.tile_pool(name="sb2", bufs=3) as sb2, \
             tc.tile_pool(name="ps_dev", bufs=2, space="PSUM") as ps_dev, \
             tc.tile_pool(name="ps_out", bufs=2, space="PSUM") as ps_out:

            # weights on the ACT ring: ScalarE is idle at startup, and this
            # keeps the SP ring free so block 0's loads start immediately
            wid_t = wts.tile([128, 128], dt.float32r)
            nc.scalar.dma_start(wid_t[:], wid_d[:])
            wnb_t = wts.tile([128, 128], dt.bfloat16)
            nc.scalar.dma_start(wnb_t[:], wnb_d[:])
            wpb_t = wts.tile([124, 128], dt.bfloat16)
            nc.scalar.dma_start(wpb_t[:], wpb_d[:])

            state = {}

            def phase0(i):
                r0, rows, _, _ = block_geom(i)
                # pane layout per partition: [x img0 | y img0 | x img1 | y img1]
                # (img-major, so each image's phase2/3 chain starts as soon as
                # its own panes are drained). All loads on the SP ring: SP
                # issues nothing else, so load triggers never queue behind
                # compute (ScalarE-issued loads measurably stall the pipeline).
                # One plain 0.5MB DMA per pane, img0 first, so a per-image
                # scan of block 0 can start at half-load.
                xy_t = sb4.tile([128, 4 * W], dt.float32r, tag="xy_t")
                for pane, (t_d, ti) in enumerate(
                        ((x_d, 0), (y_d, 0), (x_d, 1), (y_d, 1))):
                    nc.sync.dma_start(xy_t[0:rows, pane * W:(pane + 1) * W],
                                      t_d[ti, r0:r0 + rows, :])
                state[("xy", i)] = xy_t

            def phase1_scan(i):
                r0, rows, d_rows, _ = block_geom(i)
                xy_t = state.pop(("xy", i))

                # one BOX5 covers all four panes (the prefix-scan identity is
                # exact at every offset; seam outputs are simply unused). For
                # the first and last block, scan per image instead: block 0's
                # img0 scan starts at half-load (shorter fill), and the last
                # block's img0 chain starts without waiting on img1 (tail).
                hxy = sb3.tile([128, 4 * W - 4], dt.bfloat16, tag="hxy")
                if i == 0 or i == n_total - 1:
                    _hsum_into(nc, box5, sb3, hxy, xy_t, rows, 2 * W, "im0")
                    _hsum_into(nc, box5, sb3, hxy, xy_t, rows, 2 * W, "im1",
                               o0=2 * W, s0c=2 * W)
                else:
                    _hsum_into(nc, box5, sb3, hxy, xy_t, rows, 4 * W, "xy")
                state[("hxy", i)] = (xy_t, hxy)

            def phase1_mm(i):
                r0, rows, d_rows, _ = block_geom(i)
                xy_t, hxy = state.pop(("hxy", i))

                # PSUM = (25/32)*xc - box2D(x)/32 per pane: one [128,1024]
                # two-bank psum tile per pane, 2 col groups each, drained by a
                # single ScalarE copy (f32 -> bf16) as soon as ready.
                # panes (img-major): 0=x img0, 1=y img0, 2=x img1, 3=y img1;
                # per-image dev tiles so image 0's phase2 starts after only
                # its own two drains.
                xd0 = sb3.tile([128, WD], dt.bfloat16, tag="xd0")
                yd0 = sb3.tile([128, WD], dt.bfloat16, tag="yd0")
                xd1 = sb3.tile([128, WD], dt.bfloat16, tag="xd1")
                yd1 = sb3.tile([128, WD], dt.bfloat16, tag="yd1")
                dst_by_pane = (xd0, yd0, xd1, yd1)
                for pair_base in (0, 2):
                    tiles = {}
                    for pane in (pair_base, pair_base + 1):
                        src0 = pane * W
                        ps_t = ps_dev.tile([128, 1024], dt.float32, tag="devps")
                        for c0, cn in ((0, 512), (512, WD - 512)):
                            nc.tensor.matmul(ps_t[:, c0:c0 + cn],
                                             lhsT=wid_t[0:rows, :],
                                             rhs=xy_t[0:rows, src0 + 2 + c0:src0 + 2 + c0 + cn],
                                             start=True, stop=False)
                        tiles[pane] = ps_t
                    for pane in (pair_base, pair_base + 1):
                        src0 = pane * W
                        ps_t = tiles[pane]
                        for c0, cn in ((0, 512), (512, WD - 512)):
                            nc.tensor.matmul(ps_t[:, c0:c0 + cn],
                                             lhsT=wnb_t[0:rows, :],
                                             rhs=hxy[0:rows, src0 + c0:src0 + c0 + cn],
                                             start=False, stop=True)
                        nc.scalar.copy(dst_by_pane[pane][0:d_rows, :],
                                       ps_t[0:d_rows, 0:WD])
                state[i] = (xd0, yd0, xd1, yd1)

            def phase2(i):
                _, _, d_rows, _ = block_geom(i)
                xd0, yd0, xd1, yd1 = state.pop(i)
                # per image: p = xd*yd and q[c] = p[c] + p[c+2] (bf16 2x DVE)
                pq = []
                for di, (xd_t, yd_t) in enumerate(((xd0, yd0), (xd1, yd1))):
                    p_t = sb2.tile([128, WD], dt.bfloat16, tag=f"p{di}")
                    nc.vector.tensor_mul(p_t[0:d_rows, :], xd_t[0:d_rows, :],
                                         yd_t[0:d_rows, :])
                    q_t = sb2.tile([128, WD - 2], dt.bfloat16, tag=f"q{di}")
                    nc.vector.tensor_add(q_t[0:d_rows, :],
                                         p_t[0:d_rows, 0:WD - 2],
                                         p_t[0:d_rows, 2:WD])
                    pq.append((p_t, q_t))
                state[i] = pq

            def phase3(i):
                r0, _, d_rows, o_rows = block_geom(i)
                pq = state.pop(i)
                for di, (p_t, q_t) in enumerate(pq):
                    out_ps = ps_out.tile([128, 1024], dt.float32, tag="out_ps")
                    # horizontal 5-sum = q@0 + q@1 + p@4; vertical band via wpb
                    # (matmul PSUM writes must stay within one 2KB bank -> two
                    # column groups of <=512)
                    for g0, gn in ((0, 512), (512, WO - 512)):
                        for sidx, (src, off) in enumerate(
                                ((q_t, 0), (q_t, 1), (p_t, 4))):
                            nc.tensor.matmul(out_ps[:, g0:g0 + gn],
                                             lhsT=wpb_t[0:d_rows, :],
                                             rhs=src[0:d_rows, off + g0:off + g0 + gn],
                                             start=(sidx == 0), stop=(sidx == 2))
                    o_t = sb2.tile([128, WO], dt.float32, tag=f"o{di}")
                    nc.scalar.activation(o_t[0:o_rows, :],
                                         out_ps[0:o_rows, 0:WO],
                                         mybir.ActivationFunctionType.Copy,
                                         scale=OUT_SCALE)
                    nc.scalar.dma_start(o_d[di, r0:r0 + o_rows, :],
                                        o_t[0:o_rows, :])

            # software-pipelined emission: DMA prefetch runs 2 iterations
            # ahead; phase2/3 trail their producer by 1. Natural phase order:
            # the scheduler is priority-(emission-)ordered, so the scan of
            # block i must carry higher priority than block i-1's phase2/3
            # or every downstream chain queues behind the next block's scan.
            # emission = scheduler priority: the scan of block i-2 stays ahead
            # of older blocks' phase2/3 on Vector (v7 lesson), but the
            # drain-side work (ACT copies/stores, PE out-matmuls) of older
            # blocks is emitted BEFORE block i-2's dev matmuls + drains, so
            # ScalarE never head-of-line blocks on drains that PE hasn't
            # produced yet.
            for i in range(n_total + 4):
                if i < n_total:
                    phase0(i)
                if 2 <= i < n_total + 2:
                    phase1_scan(i - 2)
                if 3 <= i < n_total + 3:
                    phase2(i - 3)
                if 4 <= i < n_total + 4:
                    phase3(i - 4)
                if 2 <= i < n_total + 2:
                    phase1_mm(i - 2)

    nc.compile()
    return nc


_NC = None


def _get_nc():
    global _NC
    if _NC is None:
        _NC = build_bass()
    return _NC


def kernel(x: np.ndarray, y: np.ndarray, mean_mask: np.ndarray = None, *,
           trace: bool = False, **_ignored):
    """Full inputs x,y [16,1,1024,1024] f32 -> full output [16,1,1016,1016] f32."""
    assert x.shape == (16, 1, H, W) and y.shape == (16, 1, H, W)
    nc = _get_nc()
    wid, wnb, wpb = _make_weights()
    x4 = np.ascontiguousarray(x[:, 0], dtype=np.float32)
    y4 = np.ascontiguousarray(y[:, 0], dtype=np.float32)
    in_maps = []
    for c in range(N_CORES):
        in_maps.append({
            "x": x4[c * B_PER_CORE:(c + 1) * B_PER_CORE],
            "y": y4[c * B_PER_CORE:(c + 1) * B_PER_CORE],
            "wid": wid, "wnb": wnb, "wpb": wpb,
        })
    kw = {}
    if trace:
        kw = dict(trace=True, trace_cores=[0])
    res = bass_utils.run_bass_kernel_spmd(nc, in_maps, core_ids=list(range(N_CORES)),
                                          **kw)
    out = np.concatenate([r["o"] for r in res.results], axis=0)
    kernel.last_results = res
    return out.reshape(16, 1, HO, WO)


# revision 35
# speedup vs baseline: 1.0200x; 1.0200x over previous
"""Trainium2 Bass kernel for nn_CovarianceLayer: local 5x5 covariance of two images.

reference:
    xc = x[:, :, 2:-2, 2:-2]; yc likewise
    x_dev = xc - box5x5(x)/25 ; y_dev = yc - box5x5(y)/25
    out   = box5x5(x_dev * y_dev)/25            # [B,1,1016,1016]

Strategy (pure data parallel over batch, 2 images per NeuronCore, 8 cores):
  Per 128-row block (stride 120, 8-row vertical halo):
    - horizontal 5-tap box sums of x,y on DVE via a custom prefix-scan op
    - vertical 5-tap conv + center-crop subtraction fused into PE matmuls
      (f32r, full-rate): PSUM = Wid^T @ x_shift  -  (band/32)^T @ hx
    - ScalarE drains dev PSUM to bf16 SBUF (one [*,1020] drain per pane)
    - p = x_dev * y_dev and q[c] = p[c] + p[c+2] on DVE (bf16, 2x mode)
    - final 5x5 box: horizontal 5-sum decomposed as q@0 + q@1 + p@4, each
      vertically convolved by a band matmul accumulating into one PSUM
    - PSUM -> SBUF copies on ScalarE; loads on SP HWDGE ring, stores on the
      ACT ring (so stores never queue ahead of prefetch loads)
"""

import numpy as np

import concourse.bacc as bacc
import concourse.mybir as mybir
import concourse.tile as tile
import concourse.dve_ops as dve_ops
from concourse.dve_spec import Spec, Src0, Src1, C0, scan, AluOp, lower
from concourse.dve_uop import DveOpSpec
from concourse.dve_ops import DveOp
from concourse import bass_utils

dt = mybir.dt

H = W = 1024
HO = WO = 1016   # output spatial dims
HD = WD = 1020   # x_dev dims
B_PER_CORE = 2
N_CORES = 8
BLK = 120        # output rows per block


def _register_box5():
    """out[p,k] = sum_{d=0..4} v[p,k+d]; in0=v[:,4:4+N], in1=v[:,0:N], s0=sum(v[:,0:4])."""
    name = "BOX5_ANT"
    for op in dve_ops.OPS:
        if op.name == name:
            return op
    body = scan(AluOp.ADD, Src0 - Src1, init=C0) + Src1

    def ref(in0, in1, c0, c1, c2):
        return np.cumsum(in0 - in1, axis=-1, dtype=np.float32) + in1 + c0

    spec = Spec(body=body, reference=ref)
    row = dve_ops._CUSTOM_DVE_ROW_BASE + len(dve_ops.OPS)
    shas = {}
    for ver in ("v3", "v4"):
        uops = lower(spec, ver=ver)
        shas[ver] = DveOpSpec(name=name, opcode=row, uops=uops, rd1_en=True).sha(ver)
    op = DveOp(name, spec, subdim=False, uops_sha=shas)
    dve_ops.OPS.append(op)
    dve_ops.CUSTOM_DVE_SPECS[name] = spec
    dve_ops._SUB_OPCODE_FOR_NAME[name] = row
    return op


# psum_dev = (25/32)*xc - box2D(x)/32 = (25/32) * x_dev  (exact weights)
# p = (25/32)^2 * x_dev*y_dev (bf16); out' = box2D(p)/16; out = out' * OUT_SCALE
WID_V = 25.0 / 32.0
OUT_SCALE = 16.0 / (25.0 * WID_V * WID_V)  # = 1.048576


def _make_weights():
    import ml_dtypes
    # Wid[k, m] = 25/32 iff k == m+2   (center-crop tap, same PSUM accum group)
    # Wnb[k, m] = -1/32 iff m <= k <= m+4  (negated vertical band, bf16)
    # Wpb[k, m] = +1/16 iff m <= k <= m+4  (final vertical band, bf16)
    wid = np.zeros((128, 128), dtype=np.float32)
    wnb = np.zeros((128, 128), dtype=np.float32)
    for m in range(124):
        wid[m + 2, m] = WID_V
        wnb[m:m + 5, m] = -1.0 / 32.0
    wpb = np.zeros((124, 128), dtype=np.float32)
    for m in range(120):
        wpb[m:m + 5, m] = 1.0 / 16.0
    return wid, wnb.astype(ml_dtypes.bfloat16), wpb.astype(ml_dtypes.bfloat16)


def _hsum_into(nc, box5, sb, out_tile, src_tile, rows, n_in, tag, o0=0, s0c=0):
    """out_tile[0:rows, o0:o0+n_in-4] = horizontal 5-tap box sums of
    src_tile[0:rows, s0c:s0c+n_in]."""
    s3 = sb.tile([128, 1], dt.float32, tag=f"s3_{tag}")
    nc.vector.tensor_reduce(s3[0:rows, :], src_tile[0:rows, s0c:s0c + 4],
                            op=mybir.AluOpType.add, axis=mybir.AxisListType.X)
    n_out = n_in - 4
    nc.vector._custom_dve(box5, out=out_tile[0:rows, o0:o0 + n_out],
                          in0=src_tile[0:rows, s0c + 4:s0c + n_in],
                          in1=src_tile[0:rows, s0c:s0c + n_out],
                          s0=s3[0:rows, :])


def build_bass():
    box5 = _register_box5()
    nc = bacc.Bacc("TRN2", target_bir_lowering=False)

    x_d = nc.dram_tensor("x", [B_PER_CORE, H, W], dt.float32r, kind="ExternalInput")
    y_d = nc.dram_tensor("y", [B_PER_CORE, H, W], dt.float32r, kind="ExternalInput")
    wid_d = nc.dram_tensor("wid", [128, 128], dt.float32r, kind="ExternalInput")
    wnb_d = nc.dram_tensor("wnb", [128, 128], dt.bfloat16, kind="ExternalInput")
    wpb_d = nc.dram_tensor("wpb", [124, 128], dt.bfloat16, kind="ExternalInput")
    o_d = nc.dram_tensor("o", [B_PER_CORE, HO, WO], dt.float32, kind="ExternalOutput")

    n_blocks = (HO + BLK - 1) // BLK  # 9
    n_total = n_blocks

    def block_geom(i):
        r0 = BLK * i
        return (r0, min(128, H - r0), min(124, HD - r0), min(BLK, HO - r0))

    with tile.TileContext(nc) as tc:
        with tc.tile_pool(name="wts", bufs=1) as wts, \
             tc.tile_pool(name="sb4", bufs=4) as sb4, \
             tc.tile_pool(name="sb3", bufs=3) as sb3, \
             tc.tile_pool(name="sb2", bufs=3) as sb2, \
             tc.tile_pool(name="ps_dev", bufs=3, space="PSUM") as ps_dev, \
             tc.tile_pool(name="ps_out", bufs=2, space="PSUM") as ps_out:

            # weights on the ACT ring: ScalarE is idle at startup, and this
            # keeps the SP ring free so block 0's loads start immediately
            wid_t = wts.tile([128, 128], dt.float32r)
            nc.scalar.dma_start(wid_t[:], wid_d[:])
            wnb_t = wts.tile([128, 128], dt.bfloat16)
            nc.scalar.dma_start(wnb_t[:], wnb_d[:])
            wpb_t = wts.tile([124, 128], dt.bfloat16)
            nc.scalar.dma_start(wpb_t[:], wpb_d[:])

            state = {}

            def phase0(i):
                r0, rows, _, _ = block_geom(i)
                # free-dim layout per partition(=row): [x img0 | x img1 | y img0 | y img1]
                # one 1MB DMA per input tensor (both images), both on the SP
                # HWDGE ring; stores go on the ACT ring so a store waiting on
                # compute can never block these prefetches (FIFO per ring).
                # pane layout per partition: [x img0 | y img0 | x img1 | y img1]
                # (img-major, so each image's phase2/3 chain starts as soon as
                # its own panes are drained). All loads on the SP ring: SP
                # issues nothing else, so load triggers never queue behind
                # compute (ScalarE-issued loads measurably stall the pipeline).
                # One plain 0.5MB DMA per pane, img0 first, so a per-image
                # scan of block 0 can start at half-load.
                xy_t = sb4.tile([128, 4 * W], dt.float32r, tag="xy_t")
                for pane, (t_d, ti) in enumerate(
                        ((x_d, 0), (y_d, 0), (x_d, 1), (y_d, 1))):
                    nc.sync.dma_start(xy_t[0:rows, pane * W:(pane + 1) * W],
                                      t_d[ti, r0:r0 + rows, :])
                state[("xy", i)] = xy_t

            def phase1(i):
                r0, rows, d_rows, _ = block_geom(i)
                xy_t = state.pop(("xy", i))

                # one BOX5 covers all four panes (the prefix-scan identity is
                # exact at every offset; seam outputs are simply unused). For
                # the first and last block, scan per image instead: block 0's
                # img0 scan starts at half-load (shorter fill), and the last
                # block's img0 chain starts without waiting on img1 (tail).
                hxy = sb3.tile([128, 4 * W - 4], dt.bfloat16, tag="hxy")
                if i == 0 or i == n_total - 1:
                    _hsum_into(nc, box5, sb3, hxy, xy_t, rows, 2 * W, "im0")
                    _hsum_into(nc, box5, sb3, hxy, xy_t, rows, 2 * W, "im1",
                               o0=2 * W, s0c=2 * W)
                else:
                    _hsum_into(nc, box5, sb3, hxy, xy_t, rows, 4 * W, "xy")

                # PSUM = (25/32)*xc - box2D(x)/32 per pane: one [128,1024]
                # two-bank psum tile per pane, 2 col groups each, drained by a
                # single ScalarE copy (f32 -> bf16) as soon as ready.
                # panes (img-major): 0=x img0, 1=y img0, 2=x img1, 3=y img1;
                # per-image dev tiles so image 0's phase2 starts after only
                # its own two drains.
                xd0 = sb3.tile([128, WD], dt.bfloat16, tag="xd0")
                yd0 = sb3.tile([128, WD], dt.bfloat16, tag="yd0")
                xd1 = sb3.tile([128, WD], dt.bfloat16, tag="xd1")
                yd1 = sb3.tile([128, WD], dt.bfloat16, tag="yd1")
                dst_by_pane = (xd0, yd0, xd1, yd1)
                for pair_base in (0, 2):
                    tiles = {}
                    for pane in (pair_base, pair_base + 1):
                        src0 = pane * W
                        ps_t = ps_dev.tile([128, 1024], dt.float32, tag="devps")
                        for c0, cn in ((0, 512), (512, WD - 512)):
                            nc.tensor.matmul(ps_t[:, c0:c0 + cn],
                                             lhsT=wid_t[0:rows, :],
                                             rhs=xy_t[0:rows, src0 + 2 + c0:src0 + 2 + c0 + cn],
                                             start=True, stop=False)
                        tiles[pane] = ps_t
                    for pane in (pair_base, pair_base + 1):
                        src0 = pane * W
                        ps_t = tiles[pane]
                        for c0, cn in ((0, 512), (512, WD - 512)):
                            nc.tensor.matmul(ps_t[:, c0:c0 + cn],
                                             lhsT=wnb_t[0:rows, :],
                                             rhs=hxy[0:rows, src0 + c0:src0 + c0 + cn],
                                             start=False, stop=True)
                        nc.scalar.copy(dst_by_pane[pane][0:d_rows, :],
                                       ps_t[0:d_rows, 0:WD])
                state[i] = (xd0, yd0, xd1, yd1)

            def phase2(i):
                _, _, d_rows, _ = block_geom(i)
                xd0, yd0, xd1, yd1 = state.pop(i)
                # per image: p = xd*yd and q[c] = p[c] + p[c+2] (bf16 2x DVE)
                pq = []
                for di, (xd_t, yd_t) in enumerate(((xd0, yd0), (xd1, yd1))):
                    p_t = sb2.tile([128, WD], dt.bfloat16, tag=f"p{di}")
                    nc.vector.tensor_mul(p_t[0:d_rows, :], xd_t[0:d_rows, :],
                                         yd_t[0:d_rows, :])
                    q_t = sb2.tile([128, WD - 2], dt.bfloat16, tag=f"q{di}")
                    nc.vector.tensor_add(q_t[0:d_rows, :],
                                         p_t[0:d_rows, 0:WD - 2],
                                         p_t[0:d_rows, 2:WD])
                    pq.append((p_t, q_t))
                state[i] = pq

            def phase3(i):
                r0, _, d_rows, o_rows = block_geom(i)
                pq = state.pop(i)
                for di, (p_t, q_t) in enumerate(pq):
                    o_t = sb2.tile([128, WO], dt.float32, tag=f"o{di}")
                    # horizontal 5-sum = q@0 + q@1 + p@4; vertical band via wpb
                    # (matmul PSUM writes must stay within one 2KB bank). One
                    # single-bank psum tile per column group, drained as soon
                    # as its 3 matmuls land, keeping only 2 out-banks live so
                    # the dev psum pool can hold 3 panes in flight.
                    for g0, gn in ((0, 512), (512, WO - 512)):
                        out_ps = ps_out.tile([128, 512], dt.float32, tag="out_ps")
                        for sidx, (src, off) in enumerate(
                                ((q_t, 0), (q_t, 1), (p_t, 4))):
                            nc.tensor.matmul(out_ps[:, 0:gn],
                                             lhsT=wpb_t[0:d_rows, :],
                                             rhs=src[0:d_rows, off + g0:off + g0 + gn],
                                             start=(sidx == 0), stop=(sidx == 2))
                        nc.scalar.activation(o_t[0:o_rows, g0:g0 + gn],
                                             out_ps[0:o_rows, 0:gn],
                                             mybir.ActivationFunctionType.Copy,
                                             scale=OUT_SCALE)
                    nc.scalar.dma_start(o_d[di, r0:r0 + o_rows, :],
                                        o_t[0:o_rows, :])

            # software-pipelined emission: DMA prefetch runs 2 iterations
            # ahead; phase2/3 trail their producer by 1. Natural phase order:
            # the scheduler is priority-(emission-)ordered, so the scan of
            # block i must carry higher priority than block i-1's phase2/3
            # or every downstream chain queues behind the next block's scan.
            for i in range(n_total + 4):
                if i < n_total:
                    phase0(i)
                if 2 <= i < n_total + 2:
                    phase1(i - 2)
                if 3 <= i < n_total + 3:
                    phase2(i - 3)
                if 4 <= i < n_total + 4:
                    phase3(i - 4)

    nc.compile()
    return nc


_NC = None


def _get_nc():
    global _NC
    if _NC is None:
        _NC = build_bass()
    return _NC


def kernel(x: np.ndarray, y: np.ndarray, mean_mask: np.ndarray = None, *,
           trace: bool = False, **_ignored):
    """Full inputs x,y [16,1,1024,1024] f32 -> full output [16,1,1016,1016] f32."""
    assert x.shape == (16, 1, H, W) and y.shape == (16, 1, H, W)
    nc = _get_nc()
    wid, wnb, wpb = _make_weights()
    x4 = np.ascontiguousarray(x[:, 0], dtype=np.float32)
    y4 = np.ascontiguousarray(y[:, 0], dtype=np.float32)
    in_maps = []
    for c in range(N_CORES):
        in_maps.append({
            "x": x4[c * B_PER_CORE:(c + 1) * B_PER_CORE],
            "y": y4[c * B_PER_CORE:(c + 1) * B_PER_CORE],
            "wid": wid, "wnb": wnb, "wpb": wpb,
        })
    kw = {}
    if trace:
        kw = dict(trace=True, trace_cores=[0])
    res = bass_utils.run_bass_kernel_spmd(nc, in_maps, core_ids=list(range(N_CORES)),
                                          **kw)
    out = np.concatenate([r["o"] for r in res.results], axis=0)
    kernel.last_results = res
    return out.reshape(16, 1, HO, WO)


# revision 36
# speedup vs baseline: 1.0755x; 1.0544x over previous
"""Trainium2 Bass kernel for nn_CovarianceLayer: local 5x5 covariance of two images.

reference:
    xc = x[:, :, 2:-2, 2:-2]; yc likewise
    x_dev = xc - box5x5(x)/25 ; y_dev = yc - box5x5(y)/25
    out   = box5x5(x_dev * y_dev)/25            # [B,1,1016,1016]

Strategy (pure data parallel over batch, 2 images per NeuronCore, 8 cores):
  Per 128-row block (stride 120, 8-row vertical halo):
    - horizontal 5-tap box sums of x,y on DVE via a custom prefix-scan op
    - vertical 5-tap conv + center-crop subtraction fused into PE matmuls
      (f32r, full-rate): PSUM = Wid^T @ x_shift  -  (band/32)^T @ hx
    - ScalarE drains dev PSUM to bf16 SBUF (one [*,1020] drain per pane)
    - p = x_dev * y_dev and q[c] = p[c] + p[c+2] on DVE (bf16, 2x mode)
    - final 5x5 box: horizontal 5-sum decomposed as q@0 + q@1 + p@4, each
      vertically convolved by a band matmul accumulating into one PSUM
    - PSUM -> SBUF copies on ScalarE; loads on SP HWDGE ring, stores on the
      ACT ring (so stores never queue ahead of prefetch loads)
"""

import numpy as np

import concourse.bacc as bacc
import concourse.mybir as mybir
import concourse.tile as tile
import concourse.dve_ops as dve_ops
from concourse.dve_spec import Spec, Src0, Src1, C0, scan, AluOp, lower
from concourse.dve_uop import DveOpSpec
from concourse.dve_ops import DveOp
from concourse import bass_utils

dt = mybir.dt

H = W = 1024
HO = WO = 1016   # output spatial dims
HD = WD = 1020   # x_dev dims
B_PER_CORE = 2
N_CORES = 8
BLK = 120        # output rows per block


def _register_box5():
    """out[p,k] = sum_{d=0..4} v[p,k+d]; in0=v[:,4:4+N], in1=v[:,0:N], s0=sum(v[:,0:4])."""
    name = "BOX5_ANT"
    for op in dve_ops.OPS:
        if op.name == name:
            return op
    body = scan(AluOp.ADD, Src0 - Src1, init=C0) + Src1

    def ref(in0, in1, c0, c1, c2):
        return np.cumsum(in0 - in1, axis=-1, dtype=np.float32) + in1 + c0

    spec = Spec(body=body, reference=ref)
    row = dve_ops._CUSTOM_DVE_ROW_BASE + len(dve_ops.OPS)
    shas = {}
    for ver in ("v3", "v4"):
        uops = lower(spec, ver=ver)
        shas[ver] = DveOpSpec(name=name, opcode=row, uops=uops, rd1_en=True).sha(ver)
    op = DveOp(name, spec, subdim=False, uops_sha=shas)
    dve_ops.OPS.append(op)
    dve_ops.CUSTOM_DVE_SPECS[name] = spec
    dve_ops._SUB_OPCODE_FOR_NAME[name] = row
    return op


# psum_dev = (25/32)*xc - box2D(x)/32 = (25/32) * x_dev  (exact weights)
# p = (25/32)^2 * x_dev*y_dev (bf16); out' = box2D(p)/16; out = out' * OUT_SCALE
WID_V = 25.0 / 32.0
OUT_SCALE = 16.0 / (25.0 * WID_V * WID_V)  # = 1.048576


def _make_weights():
    import ml_dtypes
    # Wid[k, m] = 25/32 iff k == m+2   (center-crop tap, same PSUM accum group)
    # Wnb[k, m] = -1/32 iff m <= k <= m+4  (negated vertical band, bf16)
    # Wpb[k, m] = +1/16 iff m <= k <= m+4  (final vertical band, bf16)
    wid = np.zeros((128, 128), dtype=np.float32)
    wnb = np.zeros((128, 128), dtype=np.float32)
    for m in range(124):
        wid[m + 2, m] = WID_V
        wnb[m:m + 5, m] = -1.0 / 32.0
    wpb = np.zeros((124, 128), dtype=np.float32)
    for m in range(120):
        wpb[m:m + 5, m] = 1.0 / 16.0
    return wid, wnb.astype(ml_dtypes.bfloat16), wpb.astype(ml_dtypes.bfloat16)


def _hsum_into(nc, box5, sb, out_tile, src_tile, rows, n_in, tag, o0=0, s0c=0):
    """out_tile[0:rows, o0:o0+n_in-4] = horizontal 5-tap box sums of
    src_tile[0:rows, s0c:s0c+n_in]."""
    s3 = sb.tile([128, 1], dt.float32, tag=f"s3_{tag}")
    nc.vector.tensor_reduce(s3[0:rows, :], src_tile[0:rows, s0c:s0c + 4],
                            op=mybir.AluOpType.add, axis=mybir.AxisListType.X)
    n_out = n_in - 4
    nc.vector._custom_dve(box5, out=out_tile[0:rows, o0:o0 + n_out],
                          in0=src_tile[0:rows, s0c + 4:s0c + n_in],
                          in1=src_tile[0:rows, s0c:s0c + n_out],
                          s0=s3[0:rows, :])


def build_bass():
    box5 = _register_box5()
    nc = bacc.Bacc("TRN2", target_bir_lowering=False)

    x_d = nc.dram_tensor("x", [B_PER_CORE, H, W], dt.float32r, kind="ExternalInput")
    y_d = nc.dram_tensor("y", [B_PER_CORE, H, W], dt.float32r, kind="ExternalInput")
    wid_d = nc.dram_tensor("wid", [128, 128], dt.float32r, kind="ExternalInput")
    wnb_d = nc.dram_tensor("wnb", [128, 128], dt.bfloat16, kind="ExternalInput")
    wpb_d = nc.dram_tensor("wpb", [124, 128], dt.bfloat16, kind="ExternalInput")
    o_d = nc.dram_tensor("o", [B_PER_CORE, HO, WO], dt.float32, kind="ExternalOutput")

    n_blocks = (HO + BLK - 1) // BLK  # 9
    n_total = n_blocks

    def block_geom(i):
        r0 = BLK * i
        return (r0, min(128, H - r0), min(124, HD - r0), min(BLK, HO - r0))

    with tile.TileContext(nc) as tc:
        with tc.tile_pool(name="wts", bufs=1) as wts, \
             tc.tile_pool(name="sb4", bufs=4) as sb4, \
             tc.tile_pool(name="sb3", bufs=3) as sb3, \
             tc.tile_pool(name="sb2", bufs=3) as sb2, \
             tc.tile_pool(name="ps_dev", bufs=2, space="PSUM") as ps_dev, \
             tc.tile_pool(name="ps_out", bufs=2, space="PSUM") as ps_out:

            # weights on the ACT ring: ScalarE is idle at startup, and this
            # keeps the SP ring free so block 0's loads start immediately
            wid_t = wts.tile([128, 128], dt.float32r)
            nc.scalar.dma_start(wid_t[:], wid_d[:])
            wnb_t = wts.tile([128, 128], dt.bfloat16)
            nc.scalar.dma_start(wnb_t[:], wnb_d[:])
            wpb_t = wts.tile([124, 128], dt.bfloat16)
            nc.scalar.dma_start(wpb_t[:], wpb_d[:])

            state = {}

            def phase0(i):
                r0, rows, _, _ = block_geom(i)
                # free-dim layout per partition(=row): [x img0 | x img1 | y img0 | y img1]
                # one 1MB DMA per input tensor (both images), both on the SP
                # HWDGE ring; stores go on the ACT ring so a store waiting on
                # compute can never block these prefetches (FIFO per ring).
                # pane layout per partition: [x img0 | y img0 | x img1 | y img1]
                # (img-major, so each image's phase2/3 chain starts as soon as
                # its own panes are drained). All loads on the SP ring: SP
                # issues nothing else, so load triggers never queue behind
                # compute (ScalarE-issued loads measurably stall the pipeline).
                # One plain 0.5MB DMA per pane, img0 first, so a per-image
                # scan of block 0 can start at half-load.
                xy_t = sb4.tile([128, 4 * W], dt.float32r, tag="xy_t")
                for pane, (t_d, ti) in enumerate(
                        ((x_d, 0), (y_d, 0), (x_d, 1), (y_d, 1))):
                    nc.sync.dma_start(xy_t[0:rows, pane * W:(pane + 1) * W],
                                      t_d[ti, r0:r0 + rows, :])
                state[("xy", i)] = xy_t

            def phase1(i):
                r0, rows, d_rows, _ = block_geom(i)
                xy_t = state.pop(("xy", i))

                # one BOX5 covers all four panes (the prefix-scan identity is
                # exact at every offset; seam outputs are simply unused). For
                # the first and last block, scan per image instead: block 0's
                # img0 scan starts at half-load (shorter fill), and the last
                # block's img0 chain starts without waiting on img1 (tail).
                hxy = sb3.tile([128, 4 * W - 4], dt.bfloat16, tag="hxy")
                if i == 0 or i == n_total - 1:
                    _hsum_into(nc, box5, sb3, hxy, xy_t, rows, 2 * W, "im0")
                    _hsum_into(nc, box5, sb3, hxy, xy_t, rows, 2 * W, "im1",
                               o0=2 * W, s0c=2 * W)
                else:
                    _hsum_into(nc, box5, sb3, hxy, xy_t, rows, 4 * W, "xy")

                # PSUM = (25/32)*xc - box2D(x)/32 per pane: one [128,1024]
                # two-bank psum tile per pane, 2 col groups each, drained by a
                # single ScalarE copy (f32 -> bf16) as soon as ready.
                # panes (img-major): 0=x img0, 1=y img0, 2=x img1, 3=y img1;
                # per-image dev tiles so image 0's phase2 starts after only
                # its own two drains.
                xd0 = sb3.tile([128, WD], dt.bfloat16, tag="xd0")
                yd0 = sb3.tile([128, WD], dt.bfloat16, tag="yd0")
                xd1 = sb3.tile([128, WD], dt.bfloat16, tag="xd1")
                yd1 = sb3.tile([128, WD], dt.bfloat16, tag="yd1")
                dst_by_pane = (xd0, yd0, xd1, yd1)
                for pair_base in (0, 2):
                    tiles = {}
                    for pane in (pair_base, pair_base + 1):
                        src0 = pane * W
                        ps_t = ps_dev.tile([128, 1024], dt.float32, tag="devps")
                        for c0, cn in ((0, 512), (512, WD - 512)):
                            nc.tensor.matmul(ps_t[:, c0:c0 + cn],
                                             lhsT=wid_t[0:rows, :],
                                             rhs=xy_t[0:rows, src0 + 2 + c0:src0 + 2 + c0 + cn],
                                             start=True, stop=False)
                        tiles[pane] = ps_t
                    for pane in (pair_base, pair_base + 1):
                        src0 = pane * W
                        ps_t = tiles[pane]
                        for c0, cn in ((0, 512), (512, WD - 512)):
                            nc.tensor.matmul(ps_t[:, c0:c0 + cn],
                                             lhsT=wnb_t[0:rows, :],
                                             rhs=hxy[0:rows, src0 + c0:src0 + c0 + cn],
                                             start=False, stop=True)
                        nc.scalar.copy(dst_by_pane[pane][0:d_rows, :],
                                       ps_t[0:d_rows, 0:WD])
                state[i] = (xd0, yd0, xd1, yd1)

            def phase2(i):
                _, _, d_rows, _ = block_geom(i)
                xd0, yd0, xd1, yd1 = state.pop(i)
                # per image: p = xd*yd and q[c] = p[c] + p[c+2] (bf16 2x DVE)
                pq = []
                for di, (xd_t, yd_t) in enumerate(((xd0, yd0), (xd1, yd1))):
                    p_t = sb2.tile([128, WD], dt.bfloat16, tag=f"p{di}")
                    nc.vector.tensor_mul(p_t[0:d_rows, :], xd_t[0:d_rows, :],
                                         yd_t[0:d_rows, :])
                    q_t = sb2.tile([128, WD - 2], dt.bfloat16, tag=f"q{di}")
                    nc.vector.tensor_add(q_t[0:d_rows, :],
                                         p_t[0:d_rows, 0:WD - 2],
                                         p_t[0:d_rows, 2:WD])
                    pq.append((p_t, q_t))
                state[i] = pq

            def phase3(i):
                r0, _, d_rows, o_rows = block_geom(i)
                pq = state.pop(i)
                for di, (p_t, q_t) in enumerate(pq):
                    out_ps = ps_out.tile([128, 1024], dt.float32, tag="out_ps")
                    # horizontal 5-sum = q@0 + q@1 + p@4; vertical band via wpb
                    # (matmul PSUM writes must stay within one 2KB bank -> two
                    # column groups of <=512)
                    for g0, gn in ((0, 512), (512, WO - 512)):
                        for sidx, (src, off) in enumerate(
                                ((q_t, 0), (q_t, 1), (p_t, 4))):
                            nc.tensor.matmul(out_ps[:, g0:g0 + gn],
                                             lhsT=wpb_t[0:d_rows, :],
                                             rhs=src[0:d_rows, off + g0:off + g0 + gn],
                                             start=(sidx == 0), stop=(sidx == 2))
                    o_t = sb2.tile([128, WO], dt.float32, tag=f"o{di}")
                    nc.scalar.activation(o_t[0:o_rows, :],
                                         out_ps[0:o_rows, 0:WO],
                                         mybir.ActivationFunctionType.Copy,
                                         scale=OUT_SCALE)
                    nc.scalar.dma_start(o_d[di, r0:r0 + o_rows, :],
                                        o_t[0:o_rows, :])

            # software-pipelined emission: DMA prefetch runs 2 iterations
            # ahead; phase2/3 trail their producer by 1. Natural phase order:
            # the scheduler is priority-(emission-)ordered, so the scan of
            # block i must carry higher priority than block i-1's phase2/3
            # or every downstream chain queues behind the next block's scan.
            for i in range(n_total + 4):
                if i < n_total:
                    phase0(i)
                if 2 <= i < n_total + 2:
                    phase1(i - 2)
                if 3 <= i < n_total + 3:
                    phase2(i - 3)
                if 4 <= i < n_total + 4:
                    phase3(i - 4)

    nc.compile()
    return nc


_NC = None


def _get_nc():
    global _NC
    if _NC is None:
        _NC = build_bass()
    return _NC


def kernel(x: np.ndarray, y: np.ndarray, mean_mask: np.ndarray = None, *,
           trace: bool = False, **_ignored):
    """Full inputs x,y [16,1,1024,1024] f32 -> full output [16,1,1016,1016] f32."""
    assert x.shape == (16, 1, H, W) and y.shape == (16, 1, H, W)
    nc = _get_nc()
    wid, wnb, wpb = _make_weights()
    x4 = np.ascontiguousarray(x[:, 0], dtype=np.float32)
    y4 = np.ascontiguousarray(y[:, 0], dtype=np.float32)
    in_maps = []
    for c in range(N_CORES):
        in_maps.append({
            "x": x4[c * B_PER_CORE:(c + 1) * B_PER_CORE],
            "y": y4[c * B_PER_CORE:(c + 1) * B_PER_CORE],
            "wid": wid, "wnb": wnb, "wpb": wpb,
        })
    kw = {}
    if trace:
        kw = dict(trace=True, trace_cores=[0])
    res = bass_utils.run_bass_kernel_spmd(nc, in_maps, core_ids=list(range(N_CORES)),
                                          **kw)
    out = np.concatenate([r["o"] for r in res.results], axis=0)
    kernel.last_results = res
    return out.reshape(16, 1, HO, WO)
